# revision 1
# baseline (speedup 1.0000x reference)
"""3-layer GCN encoder on 8 TRN2 NeuronCores (Bass/Tile) — v2.

Structure (per layer, per core; nodes degree-rank-dealt across cores):
- g = xp @ W as 98 per-block PE matmuls (lhsT = xpT block, rhs = W) that
  emit node-major g directly into 512-col-batched PSUM chunks.
- ONE 25.7MB AllGather per layer into a core-major table (vs 2 smaller
  AGs): the collective runs at much better effective bandwidth at this
  size and pays one fixed overhead instead of two.
- Padded-CSR degree-sorted windowed gathers (signed int16 offsets from
  two window bases), pipelined group-wise with run-merged DVE segment
  reduces (adjacent equal-width blocks share one tensor_reduce).
- Per-group strided u stores pipeline under the gathers; window-0's
  alignment re-gather overlaps window-1's gather phase.
- Self-loop term added on-chip from g_nm (no gathered self descriptors).
- xp' = relu(dis2*u) per block on the scalar engine feeds the next
  layer's transposes; out = relu(dis*u) + its x_out DMA are deferred
  into the NEXT layer's AllGather window (the only idle window).
- Layer-0 front: host supplies xpT pre-scaled and pre-transposed.

Table layout: core-major, row(c, j) = c*12544 + (j%128)*98 + j//128,
T_ROWS = 100352 (+256 slack rows). Window 1 = cores 0..3 (base 32768),
window 2 = cores 4..7 (base 82944). Gather sentinels point at in-shard
pad slots (j >= 12500, zero g, rewritten by the AG each layer) chosen so
zrel = ZROW - BASE is POSITIVE: trailing negative gather idx are treated
as end markers by the DMA ucode (the kernel hangs otherwise).

CoreSim cost-model time: 1.431 ms (v1 baseline: 2.426 ms).
"""

import sys

sys.path.insert(0, "/opt/trn_rl_repo")

import numpy as np

from concourse import bass, bacc, mybir, tile
import concourse.bass_utils as bass_utils
from concourse.masks import make_identity

# ---------------- problem constants (hardcoded per harness contract) -------
N = 100000
E = 1600000
F = 64  # feature width used everywhere (W3 zero-padded 32->64)
OC = 32
NCORES = 8
KB = 98  # blocks per core
SH = KB * 128  # 12544 rows per core shard
NPC = 12500  # real nodes per core

# table layout: core-major, row(c, j) = c*SH + (j%128)*KB + j//128
P2START = 4 * SH  # 50176: first row owned by cores 4..7
T_ROWS = NCORES * SH  # 100352
BASE_1 = 32768  # window1 reach [0, 65535] covers cores 0..3
BASE_2 = 82944  # window2 reach [50176, 115711] covers cores 4..7
# zero sentinel rows: pad slot j=12543 (k=97, p=127) of core 2 / core 7.
# Chosen so zrel = ZROW - BASE is POSITIVE: trailing negative gather idx
# are treated as end markers by the DMA ucode (kernel hangs otherwise).
ZROW_1 = 2 * SH + 127 * KB + 97  # 37631 -> zrel1 = +4863
ZROW_2 = 7 * SH + 127 * KB + 97  # 100351 -> zrel2 = +17407

GMAX_COLS = 70  # max slot-columns per gather group (SWDGE ring caps ~12.5k idx/gather)

f32 = mybir.dt.float32
i16 = mybir.dt.int16

_CACHE = {}


# ============================ host preprocessing ===========================

def _wrap_idx(vals: np.ndarray) -> np.ndarray:
    """[n] int -> [128, n//16] int16 (wrapped in 16 partitions, replicated x8)."""
    n = len(vals)
    assert n % 16 == 0
    a = vals.reshape(n // 16, 16).T.astype(np.int16)
    return np.tile(a, (8, 1))


def _preprocess(edge_index: np.ndarray):
    src = np.asarray(edge_index[0], dtype=np.int64)
    dst = np.asarray(edge_index[1], dtype=np.int64)
    deg = np.bincount(dst, minlength=N).astype(np.int64) + 1

    # global degree-rank deal: rank r -> core r%8, in-core rank j=r//8
    order = np.argsort(-deg, kind="stable")  # rank -> node
    node_core = np.empty(N, np.int32)
    node_j = np.empty(N, np.int32)
    ranks = np.arange(N)
    node_core[order] = (ranks % NCORES).astype(np.int32)
    node_j[order] = (ranks // NCORES).astype(np.int32)

    # in-core coords: j -> (k=j//128, p=j%128); table row (core-major):
    #   c*SH + p*KB + k
    k_of = node_j // 128
    p_of = node_j % 128
    trow = (node_core.astype(np.int64) * SH + p_of * KB + k_of).astype(np.int64)

    # incoming edge lists grouped by dst
    eorder = np.argsort(dst, kind="stable")
    src_sorted = src[eorder]
    counts = np.bincount(dst, minlength=N)
    starts = np.zeros(N + 1, np.int64)
    np.cumsum(counts, out=starts[1:])

    # per node: incoming srcs' table rows, split by window (NO self row:
    # the self term is added on-chip from g_nm)
    all_rows = trow[src_sorted]
    w_of_row = (all_rows >= P2START).astype(np.int8)  # 0 = window1, 1 = window2

    # per-(node, window) counts
    cnt_w = np.zeros((N, 2), np.int32)
    np.add.at(cnt_w, (dst[eorder], w_of_row.astype(np.int64)), 1)

    cores = []
    percore = []
    for c in range(NCORES):
        nodes_c = order[c::NCORES]  # in-core rank j -> node
        percore.append(nodes_c)
    # shared block schedule D[w][k]: max over cores of sorted per-window counts
    D = np.zeros((2, KB), np.int32)
    pi_w = []  # [w][c] -> permutation over in-core slots (j indices incl pads)
    for w in range(2):
        perms = []
        for c in range(NCORES):
            nodes_c = percore[c]
            cw = np.zeros(SH, np.int32)
            cw[: len(nodes_c)] = cnt_w[nodes_c, w]
            perm = np.argsort(-cw, kind="stable")  # slot j' -> in-core rank j
            perms.append(perm)
            sorted_c = cw[perm]
            blk_max = sorted_c.reshape(KB, 128)[:, 0]
            D[w] = np.maximum(D[w], blk_max)
        pi_w.append(perms)
    D = np.maximum(D, 1)

    # group packing: blocks -> groups with <= GMAX_COLS slot columns
    groups = [[], []]  # [w] -> list of list-of-block-ids
    for w in range(2):
        cur, cur_cols = [], 0
        for kb in range(KB):
            d = int(D[w][kb])
            if cur and cur_cols + d > GMAX_COLS:
                groups[w].append(cur)
                cur, cur_cols = [], 0
            cur.append(kb)
            cur_cols += d
        if cur:
            groups[w].append(cur)

    zrel = (ZROW_1 - BASE_1, ZROW_2 - BASE_2)
    base_w = (BASE_1, BASE_2)

    # build per-core idx blobs + align idx + per-core arrays (vectorized)
    idx_blob = [[None] * NCORES, [None] * NCORES]
    al_idx = [[None] * NCORES, [None] * NCORES]
    deg_arr = [None] * NCORES
    dst_sorted = dst[eorder]
    core_of_dst = node_core[dst_sorted]
    j_of_dst = node_j[dst_sorted].astype(np.int64)
    w_of_e = w_of_row.astype(np.int64)
    for c in range(NCORES):
        nodes_c = percore[c]
        nc_nodes = len(nodes_c)
        cmask = core_of_dst == c
        for w in range(2):
            perm = pi_w[w][c]
            inv = np.empty(SH, np.int64)
            inv[perm] = np.arange(SH)

            # per-block absolute column bases across the whole window blob
            colbase = np.empty(KB, np.int64)
            group_cols = []
            pos = 0
            for g in groups[w]:
                for kb in g:
                    colbase[kb] = pos
                    pos += int(D[w][kb])
                pos += 1  # sentinel col
                group_cols.append(pos)
            total_cols = pos

            # edges of this (core, window): slot, ordinal-within-slot, value
            em = cmask & (w_of_e == w)
            je = j_of_dst[em]            # in-core rank of dst
            se = inv[je]                 # slot index
            rows_e = all_rows[em]        # table row of src
            order_e = np.argsort(se, kind="stable")
            se_s = se[order_e]
            rows_s = rows_e[order_e]
            # ordinal within each slot run
            cnts = np.bincount(se_s, minlength=SH)
            run_starts = np.zeros(SH + 1, np.int64)
            np.cumsum(cnts, out=run_starts[1:])
            ords = np.arange(len(se_s)) - run_starts[se_s]

            M = np.full((128, total_cols), zrel[w], np.int64)
            pe = se_s % 128
            ke = se_s // 128
            M[pe, colbase[ke] + ords] = rows_s - base_w[w]

            # wrap per group, col-major positions (pos = col*128 + p)
            parts = []
            lo = 0
            for hi in group_cols:
                parts.append(_wrap_idx(M[:, lo:hi].T.ravel()))
                lo = hi
            idx_blob[w][c] = np.concatenate(parts, axis=1)

            sw = inv[np.arange(SH)]
            al = (sw % 128) * KB + sw // 128  # u_w dram row
            al_idx[w][c] = _wrap_idx(al)

        dg = np.full((128, KB), 1e30, np.float32)
        jj = np.arange(nc_nodes)
        dg[jj % 128, jj // 128] = deg[nodes_c].astype(np.float32)
        deg_arr[c] = dg

    maxg = max(
        sum(int(D[w][kb]) for kb in g) + 1 for w in range(2) for g in groups[w]
    )
    meta = dict(
        D=D,
        groups=groups,
        maxg=maxg,
        percore=percore,
        idx_blob=idx_blob,
        al_idx=al_idx,
        deg_arr=deg_arr,
    )
    return meta


# ============================ device kernel ================================

def _build(meta, with_bias: bool, reps: int = 1):
    import os

    stop = os.environ.get("KBUILD_STOP", "full")  # ag|gather|reduce|align|full
    nlayers = int(os.environ.get("KBUILD_NLAYERS", "3"))
    max_groups = int(os.environ.get("KBUILD_MAX_GROUPS", "9999"))
    skip_ag = os.environ.get("KBUILD_SKIP_AG", "") == "1"
    nwin = int(os.environ.get("KBUILD_NWIN", "2"))
    D, groups = meta["D"], meta["groups"]
    nc = bacc.Bacc("TRN2", target_bir_lowering=False, debug=False, num_devices=NCORES)

    xpt_in = nc.dram_tensor("xpt", [F, SH], f32, kind="ExternalInput")
    deg_in = nc.dram_tensor("deg", [128, KB], f32, kind="ExternalInput")
    w_in = [
        nc.dram_tensor(f"w{l}", [F, F], f32, kind="ExternalInput") for l in (1, 2, 3)
    ]
    b_in = [
        nc.dram_tensor(f"b{l}", [1, F], f32, kind="ExternalInput") for l in (1, 2, 3)
    ]
    idx_in = [
        nc.dram_tensor(
            f"idxw{w + 1}", list(meta["idx_blob"][w][0].shape), i16, kind="ExternalInput"
        )
        for w in range(2)
    ]
    al_in = [
        nc.dram_tensor(f"alw{w + 1}", [128, SH // 16], i16, kind="ExternalInput")
        for w in range(2)
    ]
    x_out = [
        nc.dram_tensor(f"x{l}o", [128, KB * F], f32, kind="ExternalOutput")
        for l in (1, 2, 3)
    ]

    with tile.TileContext(nc) as tc:
        with (
            tc.tile_pool(name="const", bufs=1) as cpool,
            tc.tile_pool(name="sbuf", bufs=2) as sb,
            tc.tile_pool(name="big", bufs=1) as bigp,
            tc.tile_pool(name="msgs", bufs=2) as msp,
            tc.tile_pool(name="psum_mm", bufs=2, space="PSUM") as ps_mm,
            tc.tile_pool(name="psum_tr", bufs=2, space="PSUM") as ps_tr,
            tc.tile_pool(name="dram", bufs=1, space="DRAM") as dr,
        ):
            # ---- layer-0 front first: xpT chunks gate layer-1 matmuls ----
            xpT = bigp.tile([F, SH], f32, tag="xpT")
            XC = SH // 4
            for c0 in range(0, SH, XC):
                nc.sync.dma_start(
                    out=xpT[:, c0 : c0 + XC], in_=xpt_in[:, c0 : c0 + XC]
                )

            # ---- constants ----
            ident = cpool.tile([128, 128], f32)
            make_identity(nc, ident[:])
            w_sb = []
            for l in range(3):
                t = cpool.tile([F, F], f32, tag=f"w{l}")
                nc.sync.dma_start(out=t[:], in_=w_in[l][:, :])
                w_sb.append(t)
            b_sb = []
            if with_bias:
                for l in range(3):
                    t = cpool.tile([1, F], f32, tag=f"b{l}")
                    nc.sync.dma_start(out=t[:], in_=b_in[l][:, :])
                    b_sb.append(t)
            deg_sb = cpool.tile([128, KB], f32)
            nc.sync.dma_start(out=deg_sb[:], in_=deg_in[:, :])
            al_sb = cpool.tile([128, 2 * SH // 16], i16, tag="al")
            nc.sync.dma_start(out=al_sb[:, 0 : SH // 16], in_=al_in[0][:, :])
            nc.sync.dma_start(out=al_sb[:, SH // 16 :], in_=al_in[1][:, :])

            # dis = rsqrt(deg), dis2 = 1/deg  (Newton-refined)
            r0 = cpool.tile([128, KB], f32, tag="r0")
            nc.vector.reciprocal(out=r0[:], in_=deg_sb[:])
            tmp = cpool.tile([128, KB], f32, tag="rt")
            nc.vector.tensor_tensor(
                out=tmp[:], in0=deg_sb[:], in1=r0[:], op=mybir.AluOpType.mult
            )
            nc.vector.tensor_scalar(
                out=tmp[:], in0=tmp[:], scalar1=-1.0, scalar2=2.0,
                op0=mybir.AluOpType.mult, op1=mybir.AluOpType.add,
            )
            dis2_sb = cpool.tile([128, KB], f32, tag="dis2")
            nc.vector.tensor_tensor(
                out=dis2_sb[:], in0=r0[:], in1=tmp[:], op=mybir.AluOpType.mult
            )
            dis_sb = cpool.tile([128, KB], f32, tag="dis")
            nc.scalar.sqrt(out=dis_sb[:], in_=dis2_sb[:])
            # one Newton step for sqrt: dis = 0.5*dis*(3 - deg*dis^2)
            s2 = cpool.tile([128, KB], f32, tag="s2")
            nc.vector.tensor_tensor(
                out=s2[:], in0=dis_sb[:], in1=dis_sb[:], op=mybir.AluOpType.mult
            )
            nc.vector.tensor_tensor(
                out=s2[:], in0=s2[:], in1=deg_sb[:], op=mybir.AluOpType.mult
            )
            nc.vector.tensor_scalar(
                out=s2[:], in0=s2[:], scalar1=-0.5, scalar2=1.5,
                op0=mybir.AluOpType.mult, op1=mybir.AluOpType.add,
            )
            nc.vector.tensor_tensor(
                out=dis_sb[:], in0=dis_sb[:], in1=s2[:], op=mybir.AluOpType.mult
            )

            # ---- DRAM scratch ----
            # +256 slack rows: keep DMA-gather reads near the tail inside
            # the allocation (mirrors v1's trailing zero tiles)
            table = dr.tile([T_ROWS + 256, F], f32)
            g_shard = dr.tile([SH, F], f32)
            u_dram = dr.tile([2 * SH, F], f32, name="u_dram2")


            def transpose_to_xpT(xp_sb):
                """xp_sb [128, KB*F] node-major -> xpT [F, SH] feature-major.
                4 transposes share one [64, 512] PSUM bank -> one batched copy."""
                TRB = 4
                for k0 in range(0, KB, TRB):
                    kn = min(TRB, KB - k0)
                    pt = ps_tr.tile([F, TRB * 128], f32, space="PSUM", tag="trF")
                    for dk in range(kn):
                        k = k0 + dk
                        nc.tensor.transpose(
                            out=pt[:, dk * 128 : (dk + 1) * 128],
                            in_=xp_sb[:, k * F : (k + 1) * F],
                            identity=ident[:],
                        )
                    nc.scalar.copy(
                        out=xpT[:, k0 * 128 : (k0 + kn) * 128], in_=pt[:, : kn * 128]
                    )


            pending_out = None
            for _rep in range(reps):
                for l in range(nlayers):
                    # ---- A: g_nm = (xp @ W) node-major, per-block lhsT ----
                    # 8 blocks share one [128, 512] PSUM bank -> one batched copy
                    g_nm = bigp.tile([128, KB * F], f32, tag="gnm")
                    MMB = 8
                    for k0 in range(0, KB, MMB):
                        kn = min(MMB, KB - k0)
                        pm = ps_mm.tile([128, MMB * F], f32, space="PSUM", tag="mm")
                        for dk in range(kn):
                            k = k0 + dk
                            nc.tensor.matmul(
                                out=pm[:, dk * F : (dk + 1) * F],
                                lhsT=xpT[:, k * 128 : (k + 1) * 128],
                                rhs=w_sb[l][:],
                                start=True,
                                stop=True,
                            )
                        nc.scalar.copy(
                            out=g_nm[:, k0 * F : (k0 + kn) * F], in_=pm[:, : kn * F]
                        )

                    # ---- B: store shard + single AllGather ----
                    nc.sync.dma_start(out=g_shard[:, :], in_=g_nm[:])
                    if not skip_ag:
                        nc.gpsimd.collective_compute(
                            "AllGather",
                            mybir.AluOpType.bypass,
                            replica_groups=[list(range(NCORES))],
                            ins=[g_shard[:, :].opt()],
                            outs=[table[0:T_ROWS, :].opt()],
                        )

                    if pending_out is not None:
                        pending_out()
                        pending_out = None
                    if stop == "ag":
                        dbg = sb.tile([128, F], f32, tag="dbg")
                        nc.sync.dma_start(out=dbg[:], in_=table[0:128, :])
                        nc.sync.dma_start(out=x_out[l][:, 0:F], in_=dbg[:])
                        continue

                    # ---- C: window gathers + segment reduces ----
                    u_al = bigp.tile([128, 2 * KB, F], f32, tag="ual")
                    for w in range(nwin):
                        u_w = bigp.tile([128, KB * F], f32, tag="uw")
                        in_ap = (
                            table[BASE_1:P2START, :]
                            if w == 0
                            else table[BASE_2:, :]
                        )
                        off8 = 0
                        for g in groups[w][:max_groups]:
                            gcols = sum(int(D[w][kb]) for kb in g) + 1
                            nidx = gcols * 128
                            idx_sb = sb.tile([128, nidx // 16], i16, tag="idx")
                            nc.sync.dma_start(
                                out=idx_sb[:],
                                in_=idx_in[w][:, off8 : off8 + nidx // 16],
                            )
                            msgs = msp.tile([128, meta["maxg"], F], f32, tag="msgs")
                            nc.gpsimd.dma_gather(
                                out_ap=msgs[:, :gcols, :],
                                in_ap=in_ap,
                                idxs_ap=idx_sb[:],
                                num_idxs=nidx,
                                num_idxs_reg=nidx,
                                elem_size=F,
                                single_packet=False,
                            )
                            if stop == "gather":
                                nc.scalar.copy(
                                    out=u_w[:, 0:F],
                                    in_=msgs[:, 0, :],
                                )
                            else:
                                # one reduce per run of equal-d blocks
                                loc = 0
                                i = 0
                                while i < len(g):
                                    d = int(D[w][g[i]])
                                    m = 1
                                    while i + m < len(g) and int(D[w][g[i + m]]) == d:
                                        m += 1
                                    kb0 = g[i]
                                    nc.vector.tensor_reduce(
                                        out=u_w[:, kb0 * F : (kb0 + m) * F],
                                        in_=msgs[:, loc : loc + m * d, :].rearrange(
                                            "p (m d) f -> p m f d", m=m
                                        ),
                                        axis=mybir.AxisListType.X,
                                        op=mybir.AluOpType.add,
                                    )
                                    loc += m * d
                                    i += m
                                kb_lo, kb_hi = g[0], g[-1] + 1
                                u_dram_w = u_dram[
                                    w * SH : (w + 1) * SH, :
                                ].rearrange("(p k) f -> p (k f)", k=KB)
                                nc.sync.dma_start(
                                    out=u_dram_w[:, kb_lo * F : kb_hi * F],
                                    in_=u_w[:, kb_lo * F : kb_hi * F],
                                )
                            off8 += nidx // 16
                        if stop == "full":
                            # two 49-block halves: the adds chain starts on
                            # half A while half B is still in flight
                            HB = 49 * 128
                            for h in range(2):
                                nc.gpsimd.dma_gather(
                                    out_ap=u_al[
                                        :, w * KB + h * 49 : w * KB + (h + 1) * 49, :
                                    ],
                                    in_ap=u_dram[w * SH : (w + 1) * SH, :],
                                    idxs_ap=al_sb[
                                        :,
                                        w * (SH // 16) + h * (HB // 16)
                                        : w * (SH // 16) + (h + 1) * (HB // 16),
                                    ],
                                    num_idxs=HB,
                                    num_idxs_reg=HB,
                                    elem_size=F,
                                    single_packet=False,
                                )
                    if stop in ("gather", "reduce"):
                        dbg = sb.tile([128, F], f32, tag="dbg")
                        nc.sync.dma_start(out=dbg[:], in_=u_dram[0:128, :])
                        nc.sync.dma_start(out=x_out[l][:, 0:F], in_=dbg[:])
                        continue

                    # ---- D: add (+ self term from g_nm), 4-way chunked ----
                    u = u_al[:, 0:KB, :].rearrange("p k f -> p (k f)")
                    u1v = u_al[:, KB : 2 * KB, :].rearrange("p k f -> p (k f)")
                    CHK = [0, 25, 49, 74, KB]
                    for c0, c1 in zip(CHK, CHK[1:]):
                        cs = slice(c0 * F, c1 * F)
                        nc.vector.tensor_tensor(
                            out=u[:, cs], in0=u[:, cs], in1=u1v[:, cs],
                            op=mybir.AluOpType.add,
                        )
                        nc.vector.tensor_tensor(
                            out=u[:, cs], in0=u[:, cs], in1=g_nm[:, cs],
                            op=mybir.AluOpType.add,
                        )
                    if stop == "align":
                        nc.sync.dma_start(out=x_out[l][:, :], in_=u)
                        continue

                    # ---- E: postproc ----
                    # out_sb reuses ual's second half: dead after the adds,
                    # WAR with next layer's align gather is naturally late
                    out_sb = u_al[:, KB : 2 * KB, :].rearrange("p k f -> p (k f)")
                    if l < 2:
                        xp = bigp.tile([128, KB * F], f32, tag="uw")
                    if not with_bias:
                        # xp = relu(dis2*u) per block on ACT (feeds mm chain);
                        # out_sb = relu(dis*u) / dis*u deferred into the next
                        # layer's AllGather window (emit_out below)
                        if l < 2:
                            for k in range(KB):
                                cs = slice(k * F, (k + 1) * F)
                                nc.scalar.activation(
                                    out=xp[:, cs], in_=u[:, cs],
                                    func=mybir.ActivationFunctionType.Relu,
                                    scale=dis2_sb[:, k : k + 1],
                                )

                        def emit_out(l=l, u=u, out_sb=out_sb):
                            for c0, c1 in zip(CHK, CHK[1:]):
                                for k in range(c0, c1):
                                    cs = slice(k * F, (k + 1) * F)
                                    if l < 2:
                                        nc.scalar.activation(
                                            out=out_sb[:, cs], in_=u[:, cs],
                                            func=mybir.ActivationFunctionType.Relu,
                                            scale=dis_sb[:, k : k + 1],
                                        )
                                    else:
                                        nc.scalar.mul(
                                            out=out_sb[:, cs], in_=u[:, cs],
                                            mul=dis_sb[:, k : k + 1],
                                        )
                                nc.sync.dma_start(
                                    out=x_out[l][:, c0 * F : c1 * F],
                                    in_=out_sb[:, c0 * F : c1 * F],
                                )

                        if l == nlayers - 1:
                            # last layer: no AG window follows — emit inline,
                            # chunk-pipelined with the adds
                            emit_out()
                        else:
                            pending_out = emit_out
                    else:
                        # v = dis*u ; t = relu(v + b) (layers 1,2) / t = v + b (layer 3)
                        # out = t ; x' = dis*t
                        v = out_sb
                        for k in range(KB):
                            s = slice(k * F, (k + 1) * F)
                            nc.scalar.mul(
                                out=v[:, s], in_=u[:, s], mul=dis_sb[:, k : k + 1]
                            )
                        bb = b_sb[l][:].to_broadcast([128, F])
                        for k in range(KB):
                            s = slice(k * F, (k + 1) * F)
                            nc.vector.tensor_tensor(
                                out=out_sb[:, s], in0=v[:, s], in1=bb, op=mybir.AluOpType.add
                            )
                        if l < 2:
                            nc.scalar.activation(
                                out=out_sb[:],
                                in_=out_sb[:],
                                func=mybir.ActivationFunctionType.Relu,
                            )
                            for k in range(KB):
                                s = slice(k * F, (k + 1) * F)
                                nc.scalar.mul(
                                    out=xp[:, s],
                                    in_=out_sb[:, s],
                                    mul=dis_sb[:, k : k + 1],
                                )

                        def emit_out(l=l, out_sb=out_sb):
                            nc.sync.dma_start(out=x_out[l][:, :], in_=out_sb[:])

                        pending_out = emit_out

                    # ---- F: next-layer x'^T ----
                    if l < 2:
                        transpose_to_xpT(xp)
            if pending_out is not None:
                pending_out()
                pending_out = None

    nc.compile()
    return nc


# ============================ entry point =================================

def _get_compiled(edge_index, biases_zero, reps):
    key = ("k", int(np.asarray(edge_index).sum() & 0xFFFFFFF), biases_zero, reps)
    if key not in _CACHE:
        meta = _preprocess(np.asarray(edge_index))
        nc = _build(meta, with_bias=not biases_zero, reps=reps)
        _CACHE[key] = (meta, nc)
    return _CACHE[key]


def _prepare(x, edge_index, W1, b1, W2, b2, W3, b3, _reps=1):
    x = np.asarray(x, np.float32)
    biases_zero = all(
        float(np.abs(np.asarray(b)).max()) == 0.0 for b in (b1, b2, b3)
    )
    meta, nc = _get_compiled(edge_index, biases_zero, _reps)
    percore, deg_arr = meta["percore"], meta["deg_arr"]

    W3p = np.zeros((F, F), np.float32)
    W3p[:, :OC] = np.asarray(W3, np.float32)
    b3p = np.zeros((F,), np.float32)
    b3p[:OC] = np.asarray(b3, np.float32)
    Ws = [np.asarray(W1, np.float32), np.asarray(W2, np.float32), W3p]
    bs = [
        np.asarray(b1, np.float32).reshape(1, F),
        np.asarray(b2, np.float32).reshape(1, F),
        b3p.reshape(1, F),
    ]

    deg_full = np.bincount(np.asarray(edge_index[1], np.int64), minlength=N).astype(
        np.float64
    ) + 1.0
    dis_h = 1.0 / np.sqrt(deg_full)
    in_maps = []
    for c in range(NCORES):
        nodes_c = percore[c]
        xpt = np.zeros((F, SH), np.float32)
        xpt[:, : len(nodes_c)] = (
            x[nodes_c] * dis_h[nodes_c, None]
        ).T.astype(np.float32)
        m = {
            "xpt": xpt,
            "deg": deg_arr[c],
            "w1": Ws[0], "w2": Ws[1], "w3": Ws[2],
            "b1": bs[0], "b2": bs[1], "b3": bs[2],
            "idxw1": meta["idx_blob"][0][c],
            "idxw2": meta["idx_blob"][1][c],
            "alw1": meta["al_idx"][0][c],
            "alw2": meta["al_idx"][1][c],
        }
        in_maps.append(m)

    return meta, nc, in_maps


def kernel(x, edge_index, W1, b1, W2, b2, W3, b3, _reps=1):
    meta, nc, in_maps = _prepare(x, edge_index, W1, b1, W2, b2, W3, b3, _reps)
    percore = meta["percore"]
    res = bass_utils.run_bass_kernel_spmd(nc, in_maps, core_ids=list(range(NCORES)))

    # unshard: x_out tiles [128, KB*F] -> per-node rows
    out = np.empty((N, 160), np.float32)
    for c in range(NCORES):
        nodes_c = percore[c]
        jj = np.arange(len(nodes_c))
        kk, pp = jj // 128, jj % 128
        x1 = res.results[c]["x1o"].reshape(128, KB, F)[pp, kk, :]
        x2 = res.results[c]["x2o"].reshape(128, KB, F)[pp, kk, :]
        x3 = res.results[c]["x3o"].reshape(128, KB, F)[pp, kk, :OC]
        out[nodes_c] = np.concatenate([x3, x2, x1], axis=1)
    return out



# revision 43
# speedup vs baseline: 146.2657x; 146.2657x over previous
"""3-layer GCN encoder on 8 TRN2 NeuronCores (Bass/Tile) — v5.

Structure (per layer, per core; nodes degree-rank-dealt across cores):
- g = xp @ W as 98 per-block PE matmuls (lhsT = xpT block, rhs = W) that
  emit node-major g directly into 512-col-batched PSUM chunks; the
  g_shard store is chunked 4-way so each store fires as its mm copies
  land (the AllGather starts ~7us earlier).
- ONE 25.7MB AllGather per layer into a core-major table (vs 2 smaller
  AGs): the collective runs at much better effective bandwidth at this
  size and pays one fixed overhead instead of two.
- Padded-CSR degree-sorted windowed gathers (signed int16 offsets from
  two window bases), pipelined group-wise with run-merged DVE segment
  reduces (adjacent equal-width blocks share one tensor_reduce).
  Group packing is TAPERED (small first + last groups): the first
  segment-reduce starts ~8us after the AG completes, and the window's
  final transfer doesn't gate the add/postproc tail.
- Per-group strided u stores pipeline under the gathers; alignment
  re-gathers are QUARTERED at the add-chunk boundaries so each add
  chunk starts as soon as its quarter lands. Window 2's small first
  group is hoisted into window 1's stream (own head tile) so the DVE
  reduce pipeline doesn't starve at the w1->w2 transition.
- Self-loop term added on-chip from g_nm (no gathered self descriptors).
- xp' = relu(dis2*u) = (dis2-broadcast mult + max0) per add-chunk on
  DVE — NOT 98 per-block ACT ops: the serial ACT stream was the tail
  bottleneck. PSUM->SBUF copies (transpose + mm) alternate ACT/DVE so
  neither engine's copy stream serializes the tail.
- out = relu(dis*u) + its x_out DMA are deferred into the NEXT layer's
  AllGather window (the only idle window).
- Layer-0 front: host supplies xpT pre-scaled and pre-transposed.
- Layer 1 has NO AllGather: xp(0) is the host-known input, so xtf =
  xp(0)^T (j-order columns per core chunk) is replicated to every core
  and each core builds the FULL 25.7MB layer-1 table locally — xtf
  QUARTER-chunks stream through the xpT tile (4 rotation slots so the
  SP/ACT loads run back-to-back instead of waiting each slot's mms), 98
  block matmuls per core-chunk on PE, PSUM copies alternate ACT/DVE,
  half-granular spills on Pool. The front scratch is the u_al arena
  (tag ual, idle until the window-end aligns) — NOT tag uw, which the
  reduces write: aliasing uw stalled every w1 reduce behind the last
  spill. Chunks 0-3 are emitted up front (window 1 = cores 0-3, and
  w1-gather correctness relies on Pool's in-order queue: Tile only sees
  the in_ap rows [BASE_1:P2START], the negative-offset reach below
  BASE_1 is invisible to dep tracking); chunks 4-7 interleave into the
  gather schedule. Replicated input transform = standard data
  parallelism; saves a full 284us collective.

Table layout: core-major, row(c, j) = c*12544 + (j%128)*98 + j//128,
T_ROWS = 100352 (+256 slack rows). Window 1 = cores 0..3 (base 32768),
window 2 = cores 4..7 (base 82944). Gather sentinels point at in-shard
pad slots (j >= 12500, zero g, rewritten by the AG each layer) chosen so
zrel = ZROW - BASE is POSITIVE: trailing negative gather idx are treated
as end markers by the DMA ucode (the kernel hangs otherwise).

Per-layer cost split (CoreSim): AG 283.7us for layers 2-3 (irreducible:
the collective is priced at ~95 GB/s of its 25.7MB output, on the Pool
engine, and ncfw is the real-HW ceiling too) + gather window ~119us
(DVE segment-reduce bound: E*F/128 lanes) + tail ~50us
(align->add->xp->transpose->mm chain). Attempted and rejected:
remote_dma_broadcast allgather (3.8x cheaper modeled, functionally
correct in MultiCoreSim, but NRT_EXEC_UNIT_UNRECOVERABLE on the real
backend); engine-parallel split collectives (2x cheaper modeled, but
the BIR verifier only allows CollectiveCompute on DMA/Pool engines, so
the NEFF won't compile); 32-wide layer-3 path (dma_gather requires
256B elements); identity slot order (padded-CSR volume +72%).

CoreSim cost-model time: 1.187 ms (v4: 1.250, v3: 1.383, v2: 1.431, v1: 2.426).
"""

import sys

sys.path.insert(0, "/opt/trn_rl_repo")

import numpy as np

from concourse import bass, bacc, mybir, tile
import concourse.bass_utils as bass_utils
from concourse.masks import make_identity

# ---------------- problem constants (hardcoded per harness contract) -------
N = 100000
E = 1600000
F = 64  # feature width used everywhere (W3 zero-padded 32->64)
OC = 32
NCORES = 8
KB = 98  # blocks per core
SH = KB * 128  # 12544 rows per core shard
NPC = 12500  # real nodes per core

# table layout: core-major, row(c, j) = c*SH + (j%128)*KB + j//128
P2START = 4 * SH  # 50176: first row owned by cores 4..7
T_ROWS = NCORES * SH  # 100352
BASE_1 = 32768  # window1 reach [0, 65535] covers cores 0..3
BASE_2 = 82944  # window2 reach [50176, 115711] covers cores 4..7
# zero sentinel rows: pad slot j=12543 (k=97, p=127) of core 2 / core 7.
# Chosen so zrel = ZROW - BASE is POSITIVE: trailing negative gather idx
# are treated as end markers by the DMA ucode (kernel hangs otherwise).
ZROW_1 = 2 * SH + 127 * KB + 97  # 37631 -> zrel1 = +4863
ZROW_2 = 7 * SH + 127 * KB + 97  # 100351 -> zrel2 = +17407

GMAX_COLS = 70  # max slot-columns per gather group (SWDGE ring caps ~12.5k idx/gather)
GCAP_FIRST = 16  # small first group: first reduce starts ~10us earlier
GCAP_LAST = 24  # small last group: final transfer doesn't gate the tail

f32 = mybir.dt.float32
i16 = mybir.dt.int16

_CACHE = {}


# ============================ host preprocessing ===========================

def _wrap_idx(vals: np.ndarray) -> np.ndarray:
    """[n] int -> [128, n//16] int16 (wrapped in 16 partitions, replicated x8)."""
    n = len(vals)
    assert n % 16 == 0
    a = vals.reshape(n // 16, 16).T.astype(np.int16)
    return np.tile(a, (8, 1))


def _preprocess(edge_index: np.ndarray):
    src = np.asarray(edge_index[0], dtype=np.int64)
    dst = np.asarray(edge_index[1], dtype=np.int64)
    deg = np.bincount(dst, minlength=N).astype(np.int64) + 1

    # global degree-rank deal: rank r -> core r%8, in-core rank j=r//8
    order = np.argsort(-deg, kind="stable")  # rank -> node
    node_core = np.empty(N, np.int32)
    node_j = np.empty(N, np.int32)
    ranks = np.arange(N)
    node_core[order] = (ranks % NCORES).astype(np.int32)
    node_j[order] = (ranks // NCORES).astype(np.int32)

    # in-core coords: j -> (k=j//128, p=j%128); table row (core-major):
    #   c*SH + p*KB + k
    k_of = node_j // 128
    p_of = node_j % 128
    trow = (node_core.astype(np.int64) * SH + p_of * KB + k_of).astype(np.int64)

    # incoming edge lists grouped by dst
    eorder = np.argsort(dst, kind="stable")
    src_sorted = src[eorder]
    counts = np.bincount(dst, minlength=N)
    starts = np.zeros(N + 1, np.int64)
    np.cumsum(counts, out=starts[1:])

    # per node: incoming srcs' table rows, split by window (NO self row:
    # the self term is added on-chip from g_nm)
    all_rows = trow[src_sorted]
    w_of_row = (all_rows >= P2START).astype(np.int8)  # 0 = window1, 1 = window2

    # per-(node, window) counts
    cnt_w = np.zeros((N, 2), np.int32)
    np.add.at(cnt_w, (dst[eorder], w_of_row.astype(np.int64)), 1)

    cores = []
    percore = []
    for c in range(NCORES):
        nodes_c = order[c::NCORES]  # in-core rank j -> node
        percore.append(nodes_c)
    # shared block schedule D[w][k]: max over cores of sorted per-window counts
    D = np.zeros((2, KB), np.int32)
    pi_w = []  # [w][c] -> permutation over in-core slots (j indices incl pads)
    for w in range(2):
        perms = []
        for c in range(NCORES):
            nodes_c = percore[c]
            cw = np.zeros(SH, np.int32)
            cw[: len(nodes_c)] = cnt_w[nodes_c, w]
            perm = np.argsort(-cw, kind="stable")  # slot j' -> in-core rank j
            perms.append(perm)
            sorted_c = cw[perm]
            blk_max = sorted_c.reshape(KB, 128)[:, 0]
            D[w] = np.maximum(D[w], blk_max)
        pi_w.append(perms)
    D = np.maximum(D, 1)

    # group packing: blocks -> groups with <= GMAX_COLS slot columns.
    # Tapered: a small FIRST group so the first segment-reduce starts as
    # early as possible (pipeline fill), and a small LAST group so the
    # window's final transfer doesn't gate the add/postproc tail.
    groups = [[], []]  # [w] -> list of list-of-block-ids
    for w in range(2):
        cur, cur_cols = [], 0
        for kb in range(KB):
            d = int(D[w][kb])
            cap = GCAP_FIRST if not groups[w] else GMAX_COLS
            if cur and cur_cols + d > cap:
                groups[w].append(cur)
                cur, cur_cols = [], 0
            cur.append(kb)
            cur_cols += d
        if cur:
            groups[w].append(cur)
        g = groups[w][-1]
        last_cols = sum(int(D[w][kb]) for kb in g)
        if last_cols > GCAP_LAST and len(g) > 1:
            acc, cut = 0, len(g)
            for i in range(len(g) - 1, -1, -1):
                acc += int(D[w][g[i]])
                if acc > GCAP_LAST:
                    cut = i + 1
                    break
            if 0 < cut < len(g):
                groups[w][-1:] = [g[:cut], g[cut:]]

    zrel = (ZROW_1 - BASE_1, ZROW_2 - BASE_2)
    base_w = (BASE_1, BASE_2)

    # build per-core idx blobs + align idx + per-core arrays (vectorized)
    idx_blob = [[None] * NCORES, [None] * NCORES]
    al_idx = [[None] * NCORES, [None] * NCORES]
    deg_arr = [None] * NCORES
    dst_sorted = dst[eorder]
    core_of_dst = node_core[dst_sorted]
    j_of_dst = node_j[dst_sorted].astype(np.int64)
    w_of_e = w_of_row.astype(np.int64)
    for c in range(NCORES):
        nodes_c = percore[c]
        nc_nodes = len(nodes_c)
        cmask = core_of_dst == c
        for w in range(2):
            perm = pi_w[w][c]
            inv = np.empty(SH, np.int64)
            inv[perm] = np.arange(SH)

            # per-block absolute column bases across the whole window blob
            colbase = np.empty(KB, np.int64)
            group_cols = []
            pos = 0
            for g in groups[w]:
                for kb in g:
                    colbase[kb] = pos
                    pos += int(D[w][kb])
                pos += 1  # sentinel col
                group_cols.append(pos)
            total_cols = pos

            # edges of this (core, window): slot, ordinal-within-slot, value
            em = cmask & (w_of_e == w)
            je = j_of_dst[em]            # in-core rank of dst
            se = inv[je]                 # slot index
            rows_e = all_rows[em]        # table row of src
            order_e = np.argsort(se, kind="stable")
            se_s = se[order_e]
            rows_s = rows_e[order_e]
            # ordinal within each slot run
            cnts = np.bincount(se_s, minlength=SH)
            run_starts = np.zeros(SH + 1, np.int64)
            np.cumsum(cnts, out=run_starts[1:])
            ords = np.arange(len(se_s)) - run_starts[se_s]

            M = np.full((128, total_cols), zrel[w], np.int64)
            pe = se_s % 128
            ke = se_s // 128
            M[pe, colbase[ke] + ords] = rows_s - base_w[w]

            # wrap per group, col-major positions (pos = col*128 + p)
            parts = []
            lo = 0
            for hi in group_cols:
                parts.append(_wrap_idx(M[:, lo:hi].T.ravel()))
                lo = hi
            idx_blob[w][c] = np.concatenate(parts, axis=1)

            sw = inv[np.arange(SH)]
            al = (sw % 128) * KB + sw // 128  # u_w dram row
            al_idx[w][c] = _wrap_idx(al)

        dg = np.full((128, KB), 1e30, np.float32)
        jj = np.arange(nc_nodes)
        dg[jj % 128, jj // 128] = deg[nodes_c].astype(np.float32)
        deg_arr[c] = dg

    maxg = max(
        sum(int(D[w][kb]) for kb in g) + 1 for w in range(2) for g in groups[w]
    )
    meta = dict(
        D=D,
        groups=groups,
        maxg=maxg,
        percore=percore,
        idx_blob=idx_blob,
        al_idx=al_idx,
        deg_arr=deg_arr,
    )
    return meta


# ============================ device kernel ================================

def _build(meta, with_bias: bool, reps: int = 1):
    import os

    stop = os.environ.get("KBUILD_STOP", "full")  # ag|gather|reduce|align|full
    nlayers = int(os.environ.get("KBUILD_NLAYERS", "3"))
    max_groups = int(os.environ.get("KBUILD_MAX_GROUPS", "9999"))
    skip_ag = os.environ.get("KBUILD_SKIP_AG", "") == "1"
    nwin = int(os.environ.get("KBUILD_NWIN", "2"))
    D, groups = meta["D"], meta["groups"]
    nc = bacc.Bacc("TRN2", target_bir_lowering=False, debug=False, num_devices=NCORES)

    xpt_in = nc.dram_tensor("xpt", [F, SH], f32, kind="ExternalInput")
    # full pre-scaled x^T in table-row order, REPLICATED to every core:
    # layer 1's table is built locally from it (no AllGather needed)
    xtf_in = nc.dram_tensor("xtf", [F, T_ROWS], f32, kind="ExternalInput")
    deg_in = nc.dram_tensor("deg", [128, KB], f32, kind="ExternalInput")
    w_in = [
        nc.dram_tensor(f"w{l}", [F, F], f32, kind="ExternalInput") for l in (1, 2, 3)
    ]
    b_in = [
        nc.dram_tensor(f"b{l}", [1, F], f32, kind="ExternalInput") for l in (1, 2, 3)
    ]
    idx_in = [
        nc.dram_tensor(
            f"idxw{w + 1}", list(meta["idx_blob"][w][0].shape), i16, kind="ExternalInput"
        )
        for w in range(2)
    ]
    al_in = [
        nc.dram_tensor(f"alw{w + 1}", [128, SH // 16], i16, kind="ExternalInput")
        for w in range(2)
    ]
    x_out = [
        nc.dram_tensor(f"x{l}o", [128, KB * F], f32, kind="ExternalOutput")
        for l in (1, 2, 3)
    ]

    with tile.TileContext(nc) as tc:
        with (
            tc.tile_pool(name="const", bufs=1) as cpool,
            tc.tile_pool(name="sbuf", bufs=2) as sb,
            tc.tile_pool(name="big", bufs=1) as bigp,
            tc.tile_pool(name="msgs", bufs=2) as msp,
            tc.tile_pool(name="psum_mm", bufs=4, space="PSUM") as ps_mm,
            tc.tile_pool(name="psum_tr", bufs=4, space="PSUM") as ps_tr,
            tc.tile_pool(name="dram", bufs=1, space="DRAM") as dr,
        ):
            # ---- layer-0 front first: xpT chunks gate layer-1 matmuls ----
            xpT = bigp.tile([F, SH], f32, tag="xpT")
            XC = SH // 4
            for c0 in range(0, SH, XC):
                nc.sync.dma_start(
                    out=xpT[:, c0 : c0 + XC], in_=xpt_in[:, c0 : c0 + XC]
                )

            # ---- constants ----
            ident = cpool.tile([128, 128], f32)
            make_identity(nc, ident[:])
            w_sb = []
            for l in range(3):
                t = cpool.tile([F, F], f32, tag=f"w{l}")
                nc.sync.dma_start(out=t[:], in_=w_in[l][:, :])
                w_sb.append(t)
            b_sb = []
            if with_bias:
                for l in range(3):
                    t = cpool.tile([1, F], f32, tag=f"b{l}")
                    nc.sync.dma_start(out=t[:], in_=b_in[l][:, :])
                    b_sb.append(t)
            deg_sb = cpool.tile([128, KB], f32)
            nc.sync.dma_start(out=deg_sb[:], in_=deg_in[:, :])
            al_sb = cpool.tile([128, 2 * SH // 16], i16, tag="al")
            nc.sync.dma_start(out=al_sb[:, 0 : SH // 16], in_=al_in[0][:, :])
            nc.sync.dma_start(out=al_sb[:, SH // 16 :], in_=al_in[1][:, :])

            # dis = rsqrt(deg), dis2 = 1/deg  (Newton-refined)
            r0 = cpool.tile([128, KB], f32, tag="r0")
            nc.vector.reciprocal(out=r0[:], in_=deg_sb[:])
            tmp = cpool.tile([128, KB], f32, tag="rt")
            nc.vector.tensor_tensor(
                out=tmp[:], in0=deg_sb[:], in1=r0[:], op=mybir.AluOpType.mult
            )
            nc.vector.tensor_scalar(
                out=tmp[:], in0=tmp[:], scalar1=-1.0, scalar2=2.0,
                op0=mybir.AluOpType.mult, op1=mybir.AluOpType.add,
            )
            dis2_sb = cpool.tile([128, KB], f32, tag="dis2")
            nc.vector.tensor_tensor(
                out=dis2_sb[:], in0=r0[:], in1=tmp[:], op=mybir.AluOpType.mult
            )
            dis_sb = cpool.tile([128, KB], f32, tag="dis")
            nc.scalar.sqrt(out=dis_sb[:], in_=dis2_sb[:])
            # one Newton step for sqrt: dis = 0.5*dis*(3 - deg*dis^2)
            s2 = cpool.tile([128, KB], f32, tag="s2")
            nc.vector.tensor_tensor(
                out=s2[:], in0=dis_sb[:], in1=dis_sb[:], op=mybir.AluOpType.mult
            )
            nc.vector.tensor_tensor(
                out=s2[:], in0=s2[:], in1=deg_sb[:], op=mybir.AluOpType.mult
            )
            nc.vector.tensor_scalar(
                out=s2[:], in0=s2[:], scalar1=-0.5, scalar2=1.5,
                op0=mybir.AluOpType.mult, op1=mybir.AluOpType.add,
            )
            nc.vector.tensor_tensor(
                out=dis_sb[:], in0=dis_sb[:], in1=s2[:], op=mybir.AluOpType.mult
            )

            # ---- DRAM scratch ----
            # +256 slack rows: keep DMA-gather reads near the tail inside
            # the allocation (mirrors v1's trailing zero tiles)
            table = dr.tile([T_ROWS + 256, F], f32)
            g_shard = dr.tile([SH, F], f32)
            u_dram = dr.tile([2 * SH, F], f32, name="u_dram2")


            def psum_copy(i, out, in_):
                """PSUM->SBUF copy alternating ACT / DVE so neither engine's
                copy stream serializes the tail."""
                if i % 2 == 0:
                    nc.scalar.copy(out=out, in_=in_)
                else:
                    nc.vector.tensor_scalar(
                        out=out, in0=in_, scalar1=0.0, scalar2=None,
                        op0=mybir.AluOpType.add,
                    )

            def transpose_to_xpT(xp_sb):
                """xp_sb [128, KB*F] node-major -> xpT [F, SH] feature-major.
                4 transposes share one [64, 512] PSUM bank -> one batched copy."""
                TRB = 4
                for k0 in range(0, KB, TRB):
                    kn = min(TRB, KB - k0)
                    pt = ps_tr.tile([F, TRB * 128], f32, space="PSUM", tag="trF")
                    for dk in range(kn):
                        k = k0 + dk
                        nc.tensor.transpose(
                            out=pt[:, dk * 128 : (dk + 1) * 128],
                            in_=xp_sb[:, k * F : (k + 1) * F],
                            identity=ident[:],
                        )
                    psum_copy(
                        k0 // TRB,
                        xpT[:, k0 * 128 : (k0 + kn) * 128],
                        pt[:, : kn * 128],
                    )


            pending_out = None
            for _rep in range(reps):
                for l in range(nlayers):
                    # ---- A: g_nm = (xp @ W) node-major, per-block lhsT ----
                    # 8 blocks share one [128, 512] PSUM bank -> one batched copy
                    g_nm = bigp.tile([128, KB * F], f32, tag="gnm")
                    MMB = 8
                    for k0 in range(0, KB, MMB):
                        kn = min(MMB, KB - k0)
                        pm = ps_mm.tile([128, MMB * F], f32, space="PSUM", tag="mm")
                        for dk in range(kn):
                            k = k0 + dk
                            nc.tensor.matmul(
                                out=pm[:, dk * F : (dk + 1) * F],
                                lhsT=xpT[:, k * 128 : (k + 1) * 128],
                                rhs=w_sb[l][:],
                                start=True,
                                stop=True,
                            )
                        psum_copy(
                            k0 // MMB,
                            g_nm[:, k0 * F : (k0 + kn) * F],
                            pm[:, : kn * F],
                        )

                    # ---- B: publish the table ----
                    # Layer 1: NO AllGather. xp(0) is the (host-known) input,
                    # so every core holds replicated xtf = xp(0)^T in table
                    # order and builds the FULL table locally: per core-chunk
                    # c, stream xtf half-chunks into the xpT tile (ping-pong
                    # halves; loads alternate SP/ACT), run the 98 block mms,
                    # copy PSUM->scratch, spill to the table's core-c rows.
                    # Window-1 gathers start as soon as chunks 0-3 land.
                    pending_front = []
                    if l == 0 and not skip_ag:
                        # front scratch = u_al arena (tag ual): it is not
                        # touched until the align gathers at the window END,
                        # unlike tag "uw" which the reduces write — aliasing
                        # that stalled every w1 reduce behind the last spill
                        xsc_t = bigp.tile(
                            [128, 2 * KB, F], f32, tag="ual", name="ual_front"
                        )
                        xsc = xsc_t[:, 0:KB, :].rearrange("p k f -> p (k f)")
                        HC = SH // 2

                        Q4 = [0, 25, 49, 74, KB]

                        def emit_front_chunk(c):
                            for h in range(4):
                                kb0, kb1 = Q4[h], Q4[h + 1]
                                c0, c1 = kb0 * 128, kb1 * 128
                                ld = nc.sync if h % 2 == 0 else nc.scalar
                                ld.dma_start(
                                    out=xpT[:, c0:c1],
                                    in_=xtf_in[:, c * SH + c0 : c * SH + c1],
                                )
                                for k0 in range(kb0, kb1, MMB):
                                    kn = min(MMB, kb1 - k0)
                                    pm = ps_mm.tile(
                                        [128, MMB * F], f32, space="PSUM", tag="mm"
                                    )
                                    for dk in range(kn):
                                        k = k0 + dk
                                        nc.tensor.matmul(
                                            out=pm[:, dk * F : (dk + 1) * F],
                                            lhsT=xpT[:, k * 128 : (k + 1) * 128],
                                            rhs=w_sb[0][:],
                                            start=True,
                                            stop=True,
                                        )
                                    psum_copy(
                                        k0 // MMB,
                                        xsc_t[
                                            :, k0 : k0 + kn, :
                                        ].rearrange("p k f -> p (k f)"),
                                        pm[:, : kn * F],
                                    )
                            # half-granular spills: chunk c+1's copies only
                            # wait for the half they overwrite
                            trow_c = table[c * SH : (c + 1) * SH, :].rearrange(
                                "(p k) f -> p (k f)", k=KB
                            )
                            nc.gpsimd.dma_start(
                                out=trow_c[:, : 49 * F],
                                in_=xsc_t[:, 0:49, :].rearrange(
                                    "p k f -> p (k f)"
                                ),
                            )
                            nc.gpsimd.dma_start(
                                out=trow_c[:, 49 * F :],
                                in_=xsc_t[:, 49:KB, :].rearrange(
                                    "p k f -> p (k f)"
                                ),
                            )

                        # chunks 0-3 up front (window 1 reads cores 0-3);
                        # chunks 4-7 are interleaved into the gather schedule
                        # so Pool's in-order queue alternates spill/gathers
                        for c in range(4):
                            emit_front_chunk(c)
                        pending_front = [4, 5, 6, 7]
                    else:
                        gsr = g_shard[:, :].rearrange("(p k) f -> p (k f)", k=KB)
                        for c0, c1 in ((0, 24), (24, 48), (48, 72), (72, KB)):
                            nc.sync.dma_start(
                                out=gsr[:, c0 * F : c1 * F],
                                in_=g_nm[:, c0 * F : c1 * F],
                            )
                        if not skip_ag:
                            nc.gpsimd.collective_compute(
                                "AllGather",
                                mybir.AluOpType.bypass,
                                replica_groups=[list(range(NCORES))],
                                ins=[g_shard[:, :].opt()],
                                outs=[table[0:T_ROWS, :].opt()],
                            )

                    if pending_out is not None:
                        pending_out()
                        pending_out = None
                    if stop == "ag":
                        dbg = sb.tile([128, F], f32, tag="dbg")
                        nc.sync.dma_start(out=dbg[:], in_=table[0:128, :])
                        nc.sync.dma_start(out=x_out[l][:, 0:F], in_=dbg[:])
                        continue

                    # ---- C: window gathers + segment reduces ----
                    # Flat interleaved schedule: window 2's small FIRST group
                    # is hoisted into the middle of window 1's stream (own
                    # head tile) so the DVE reduce pipeline doesn't starve at
                    # the w1->w2 transition.
                    u_al = bigp.tile([128, 2 * KB, F], f32, tag="ual")
                    in_aps = [table[BASE_1:P2START, :], table[BASE_2:, :]]
                    off8s = []
                    for w in range(2):
                        offs, pos = [], 0
                        for g in groups[w]:
                            offs.append(pos)
                            pos += (sum(int(D[w][kb]) for kb in g) + 1) * 8
                        off8s.append(offs)
                    seq = [(w, gi) for w in range(nwin) for gi in range(
                        min(len(groups[w]), max_groups))]
                    hoisted = (
                        stop == "full" and nwin == 2 and len(groups[0]) > 3
                    )
                    if hoisted:
                        item = seq.pop(len(groups[0]))  # (1, 0)
                        seq.insert(len(groups[0]) - 2, item)
                        g0len = len(groups[1][0])
                        u_head = bigp.tile([128, g0len * F], f32, tag="uwh")
                    u_ws = [None, None]

                    def emit_group(w, gi):
                        g = groups[w][gi]
                        if hoisted and w == 1 and gi == 0:
                            u_t, kb_base = u_head, 0
                        else:
                            if u_ws[w] is None:
                                u_ws[w] = bigp.tile(
                                    [128, KB * F], f32, tag="uw", name="u_w"
                                )
                            u_t, kb_base = u_ws[w], 0
                        off8 = off8s[w][gi]
                        gcols = sum(int(D[w][kb]) for kb in g) + 1
                        nidx = gcols * 128
                        idx_sb = sb.tile([128, nidx // 16], i16, tag="idx")
                        nc.sync.dma_start(
                            out=idx_sb[:],
                            in_=idx_in[w][:, off8 : off8 + nidx // 16],
                        )
                        msgs = msp.tile([128, meta["maxg"], F], f32, tag="msgs")
                        nc.gpsimd.dma_gather(
                            out_ap=msgs[:, :gcols, :],
                            in_ap=in_aps[w],
                            idxs_ap=idx_sb[:],
                            num_idxs=nidx,
                            num_idxs_reg=nidx,
                            elem_size=F,
                            single_packet=False,
                        )
                        if stop == "gather":
                            nc.scalar.copy(out=u_t[:, 0:F], in_=msgs[:, 0, :])
                            return
                        # one reduce per run of equal-d blocks
                        loc = 0
                        i = 0
                        while i < len(g):
                            d = int(D[w][g[i]])
                            m = 1
                            while i + m < len(g) and int(D[w][g[i + m]]) == d:
                                m += 1
                            kb0 = g[i] - kb_base
                            nc.vector.tensor_reduce(
                                out=u_t[:, kb0 * F : (kb0 + m) * F],
                                in_=msgs[:, loc : loc + m * d, :].rearrange(
                                    "p (m d) f -> p m f d", m=m
                                ),
                                axis=mybir.AxisListType.X,
                                op=mybir.AluOpType.add,
                            )
                            loc += m * d
                            i += m
                        kb_lo, kb_hi = g[0], g[-1] + 1
                        u_dram_w = u_dram[
                            w * SH : (w + 1) * SH, :
                        ].rearrange("(p k) f -> p (k f)", k=KB)
                        nc.sync.dma_start(
                            out=u_dram_w[:, kb_lo * F : kb_hi * F],
                            in_=u_t[:, (g[0] - kb_base) * F : (kb_hi - kb_base) * F],
                        )

                    done = [0, 0]
                    for si, (w, gi) in enumerate(seq):
                        emit_group(w, gi)
                        # interleave the remaining layer-1 front chunks so
                        # their spills don't queue ahead of the gathers on
                        # Pool (in-order engine queue)
                        if pending_front and si % 3 == 2:
                            emit_front_chunk(pending_front.pop(0))
                        done[w] += 1
                        if stop == "full" and done[w] == min(
                            len(groups[w]), max_groups
                        ):
                            # quarters at the add-chunk boundaries: each add
                            # chunk starts as soon as ITS quarter lands
                            QB = [0, 25, 49, 74, KB]
                            for q0, q1 in zip(QB, QB[1:]):
                                nq = (q1 - q0) * 128
                                nc.gpsimd.dma_gather(
                                    out_ap=u_al[:, w * KB + q0 : w * KB + q1, :],
                                    in_ap=u_dram[w * SH : (w + 1) * SH, :],
                                    idxs_ap=al_sb[
                                        :,
                                        w * (SH // 16) + q0 * 8
                                        : w * (SH // 16) + q1 * 8,
                                    ],
                                    num_idxs=nq,
                                    num_idxs_reg=nq,
                                    elem_size=F,
                                    single_packet=False,
                                )
                    if stop in ("gather", "reduce"):
                        dbg = sb.tile([128, F], f32, tag="dbg")
                        nc.sync.dma_start(out=dbg[:], in_=u_dram[0:128, :])
                        nc.sync.dma_start(out=x_out[l][:, 0:F], in_=dbg[:])
                        continue

                    # ---- D: add (+ self term from g_nm), 4-way chunked ----
                    u = u_al[:, 0:KB, :].rearrange("p k f -> p (k f)")
                    u1v = u_al[:, KB : 2 * KB, :].rearrange("p k f -> p (k f)")
                    CHK = [0, 25, 49, 74, KB]
                    for c0, c1 in zip(CHK, CHK[1:]):
                        cs = slice(c0 * F, c1 * F)
                        nc.vector.tensor_tensor(
                            out=u[:, cs], in0=u[:, cs], in1=u1v[:, cs],
                            op=mybir.AluOpType.add,
                        )
                        nc.vector.tensor_tensor(
                            out=u[:, cs], in0=u[:, cs], in1=g_nm[:, cs],
                            op=mybir.AluOpType.add,
                        )
                    if stop == "align":
                        nc.sync.dma_start(out=x_out[l][:, :], in_=u)
                        continue

                    # ---- E: postproc ----
                    # out_sb reuses ual's second half: dead after the adds,
                    # WAR with next layer's align gather is naturally late
                    out_sb = u_al[:, KB : 2 * KB, :].rearrange("p k f -> p (k f)")
                    if l < 2:
                        xp = bigp.tile([128, KB * F], f32, tag="uw")
                    if not with_bias:
                        # xp = relu(dis2*u) per CHK chunk on DVE (mult with a
                        # free-dim-broadcast dis2 + max0) — keeps the ACT
                        # engine free for the PSUM copies that follow; ACT was
                        # the serial tail bottleneck (98 x 238ns relu ops)
                        if l < 2:
                            for c0, c1 in zip(CHK, CHK[1:]):
                                nk = c1 - c0
                                cs = slice(c0 * F, c1 * F)
                                d2 = (
                                    dis2_sb[:, c0:c1]
                                    .rearrange("p (k o) -> p k o", o=1)
                                    .to_broadcast([128, nk, F])
                                )
                                nc.vector.tensor_tensor(
                                    out=xp[:, cs].rearrange(
                                        "p (k f) -> p k f", f=F
                                    ),
                                    in0=u[:, cs].rearrange(
                                        "p (k f) -> p k f", f=F
                                    ),
                                    in1=d2,
                                    op=mybir.AluOpType.mult,
                                )
                                nc.vector.tensor_scalar(
                                    out=xp[:, cs], in0=xp[:, cs],
                                    scalar1=0.0, scalar2=None,
                                    op0=mybir.AluOpType.max,
                                )

                        def emit_out(l=l, u=u, out_sb=out_sb):
                            for c0, c1 in zip(CHK, CHK[1:]):
                                for k in range(c0, c1):
                                    cs = slice(k * F, (k + 1) * F)
                                    if l < 2:
                                        nc.scalar.activation(
                                            out=out_sb[:, cs], in_=u[:, cs],
                                            func=mybir.ActivationFunctionType.Relu,
                                            scale=dis_sb[:, k : k + 1],
                                        )
                                    else:
                                        nc.scalar.mul(
                                            out=out_sb[:, cs], in_=u[:, cs],
                                            mul=dis_sb[:, k : k + 1],
                                        )
                                nc.sync.dma_start(
                                    out=x_out[l][:, c0 * F : c1 * F],
                                    in_=out_sb[:, c0 * F : c1 * F],
                                )

                        if l == nlayers - 1:
                            # last layer: no AG window follows — emit inline,
                            # chunk-pipelined with the adds
                            emit_out()
                        else:
                            pending_out = emit_out
                    else:
                        # v = dis*u ; t = relu(v + b) (layers 1,2) / t = v + b (layer 3)
                        # out = t ; x' = dis*t
                        v = out_sb
                        for k in range(KB):
                            s = slice(k * F, (k + 1) * F)
                            nc.scalar.mul(
                                out=v[:, s], in_=u[:, s], mul=dis_sb[:, k : k + 1]
                            )
                        bb = b_sb[l][:].to_broadcast([128, F])
                        for k in range(KB):
                            s = slice(k * F, (k + 1) * F)
                            nc.vector.tensor_tensor(
                                out=out_sb[:, s], in0=v[:, s], in1=bb, op=mybir.AluOpType.add
                            )
                        if l < 2:
                            nc.scalar.activation(
                                out=out_sb[:],
                                in_=out_sb[:],
                                func=mybir.ActivationFunctionType.Relu,
                            )
                            for k in range(KB):
                                s = slice(k * F, (k + 1) * F)
                                nc.scalar.mul(
                                    out=xp[:, s],
                                    in_=out_sb[:, s],
                                    mul=dis_sb[:, k : k + 1],
                                )

                        def emit_out(l=l, out_sb=out_sb):
                            nc.sync.dma_start(out=x_out[l][:, :], in_=out_sb[:])

                        pending_out = emit_out

                    # ---- F: next-layer x'^T ----
                    if l < 2:
                        transpose_to_xpT(xp)
            if pending_out is not None:
                pending_out()
                pending_out = None

    nc.compile()
    return nc


# ============================ entry point =================================

def _get_compiled(edge_index, biases_zero, reps):
    key = ("k", int(np.asarray(edge_index).sum() & 0xFFFFFFF), biases_zero, reps)
    if key not in _CACHE:
        meta = _preprocess(np.asarray(edge_index))
        nc = _build(meta, with_bias=not biases_zero, reps=reps)
        _CACHE[key] = (meta, nc)
    return _CACHE[key]


def _prepare(x, edge_index, W1, b1, W2, b2, W3, b3, _reps=1):
    x = np.asarray(x, np.float32)
    biases_zero = all(
        float(np.abs(np.asarray(b)).max()) == 0.0 for b in (b1, b2, b3)
    )
    meta, nc = _get_compiled(edge_index, biases_zero, _reps)
    percore, deg_arr = meta["percore"], meta["deg_arr"]

    W3p = np.zeros((F, F), np.float32)
    W3p[:, :OC] = np.asarray(W3, np.float32)
    b3p = np.zeros((F,), np.float32)
    b3p[:OC] = np.asarray(b3, np.float32)
    Ws = [np.asarray(W1, np.float32), np.asarray(W2, np.float32), W3p]
    bs = [
        np.asarray(b1, np.float32).reshape(1, F),
        np.asarray(b2, np.float32).reshape(1, F),
        b3p.reshape(1, F),
    ]

    deg_full = np.bincount(np.asarray(edge_index[1], np.int64), minlength=N).astype(
        np.float64
    ) + 1.0
    dis_h = 1.0 / np.sqrt(deg_full)
    # full pre-scaled x^T in TABLE-ROW order (row = c*SH + (j%128)*KB + j//128),
    # replicated to every core: layer 1 builds its table locally from this
    # j-ORDER columns per core chunk (like xpt): the front's matmul reads
    # xpT slots j = k*128+p; the spill maps (p,k) -> table row p*KB+k
    xtf = np.zeros((F, T_ROWS), np.float32)
    for c in range(NCORES):
        nodes_c = percore[c]
        jj = np.arange(len(nodes_c))
        xtf[:, c * SH + jj] = (
            x[nodes_c] * dis_h[nodes_c, None]
        ).T.astype(np.float32)
    in_maps = []
    for c in range(NCORES):
        nodes_c = percore[c]
        xpt = np.zeros((F, SH), np.float32)
        xpt[:, : len(nodes_c)] = (
            x[nodes_c] * dis_h[nodes_c, None]
        ).T.astype(np.float32)
        m = {
            "xpt": xpt,
            "xtf": xtf,
            "deg": deg_arr[c],
            "w1": Ws[0], "w2": Ws[1], "w3": Ws[2],
            "b1": bs[0], "b2": bs[1], "b3": bs[2],
            "idxw1": meta["idx_blob"][0][c],
            "idxw2": meta["idx_blob"][1][c],
            "alw1": meta["al_idx"][0][c],
            "alw2": meta["al_idx"][1][c],
        }
        in_maps.append(m)

    return meta, nc, in_maps


def kernel(x, edge_index, W1, b1, W2, b2, W3, b3, _reps=1):
    meta, nc, in_maps = _prepare(x, edge_index, W1, b1, W2, b2, W3, b3, _reps)
    percore = meta["percore"]
    res = bass_utils.run_bass_kernel_spmd(nc, in_maps, core_ids=list(range(NCORES)))

    # unshard: x_out tiles [128, KB*F] -> per-node rows
    out = np.empty((N, 160), np.float32)
    for c in range(NCORES):
        nodes_c = percore[c]
        jj = np.arange(len(nodes_c))
        kk, pp = jj // 128, jj % 128
        x1 = res.results[c]["x1o"].reshape(128, KB, F)[pp, kk, :]
        x2 = res.results[c]["x2o"].reshape(128, KB, F)[pp, kk, :]
        x3 = res.results[c]["x3o"].reshape(128, KB, F)[pp, kk, :OC]
        out[nodes_c] = np.concatenate([x3, x2, x1], axis=1)
    return out

